# revision 17
# baseline (speedup 1.0000x reference)
"""CenterLoss Trainium2 kernel (raw Bacc, hand-placed semaphores).

Math: the reference builds the full [B, C] distance matrix, masks it with a
one-hot of labels, clips to [1e-12, 1e12] and sums. Since the mask is
one-hot, only distmat[b, labels[b]] survives with its value; every other
entry contributes clip(0) = 1e-12, so

    loss = (sum_b clip(||e_b - c_{l_b}||^2, 1e-12, 1e12)) / B + (C-1)*1e-12

Batch is sharded 8 ways (512 rows/core). Per core the kernel:
  - DMAs the 512 wrapped int16 labels and the 512x256 bf16 embeddings
    (SP engine, HWDGE).
  - Gathers the 512 labelled bf16 center rows from HBM with two SWDGE
    dma_gathers (384 + 128 rows): the asymmetric split lets most of the
    DVE work overlap the second gather's DMA+semaphore latency.
  - DVE computes diff = c - e (2x-mode bf16 tensor_tensor) and
    square-accumulates per tile into rowtot[128, 4] (f32 accumulator) via
    scalar_tensor_tensor; ACT squares tile 1 in parallel.
  - SP stores rowtot[128, 4] (the per-sample squared distances).
Host side clips each of the 4096 per-sample distances to [1e-12, 1e12],
sums, divides by B and adds the (C-1)*1e-12 clamp-floor constant.

Numerics: bf16 inputs with f32 accumulation give rel err ~2e-5 vs the f32
reference (tolerance 2e-2); per-sample clip semantics are preserved
exactly on host.
"""

from contextlib import ExitStack

import numpy as np
import ml_dtypes

import concourse.bass as bass
from concourse import bacc, mybir

NUM_CLASSES = 32000
FEAT_DIM = 256
BATCH = 4096
N_CORES = 8
LAMBDA_C = 1.0
CLAMP_MIN = 1e-12
CLAMP_MAX = 1e12

P = 128
ROWS_PER_CORE = BATCH // N_CORES  # 512
TILES_PER_CORE = ROWS_PER_CORE // P  # 4
IDX_WRAP = 16
IDX_COLS = ROWS_PER_CORE // IDX_WRAP  # 32
IDX_TOTAL_COLS = IDX_COLS + 4
ROWS_A = 384  # first gather (tiles 0-2)
ROWS_B = ROWS_PER_CORE - ROWS_A  # 128 (tile 3)
TILES_A = ROWS_A // P
COLS_A = ROWS_A // IDX_WRAP  # 24

# feature flags (bisection knobs)
USE_TRIGGER_GATHERS = True
USE_KV_OUT = False
STRIP_MEMSETS = True

_nc_cache = None


def _strip_const_memsets(nc: bass.Bass) -> None:
    """Drop the const-AP registration memsets from the entry block.

    Bass.__init__ memsets four constant SBUF tiles (f32 0/1, bf16 1,
    uint8 127) on gpsimd before the entry barrier. This kernel never reads
    a const AP, and the four serialized Pool memsets delay the entry
    barrier (and with it the first DMA) by ~440ns.
    """
    entry = nc.m.functions[0].blocks[0]
    entry.instructions[:] = [
        i
        for i in entry.instructions
        if not (
            isinstance(i, mybir.InstMemset)
            # const-float32-0.0 stays: activation() reads it as the bias.
            and i.outs[0].memref != "const-float32-0.0"
        )
    ]


def build_bass(reset_sems: bool = True) -> bass.Bass:
    nc = bacc.Bacc()
    if STRIP_MEMSETS:
        _strip_const_memsets(nc)
    f32 = mybir.dt.float32
    bf16 = mybir.dt.bfloat16
    i16 = mybir.dt.int16
    i32 = mybir.dt.int32
    Alu = mybir.AluOpType

    emb = nc.declare_dram_parameter(
        "embeddings", [ROWS_PER_CORE, FEAT_DIM], bf16, isOutput=False
    )
    lab = nc.declare_dram_parameter(
        "labels", [P, IDX_TOTAL_COLS], i16, isOutput=False
    )
    cen = nc.declare_dram_parameter(
        "centers", [NUM_CLASSES, FEAT_DIM], bf16, isOutput=False
    )
    if USE_KV_OUT:
        out = nc.declare_dram_parameter(
            "partial", [1, P, TILES_PER_CORE, 1], f32, isOutput=True
        )
    else:
        out = nc.declare_dram_parameter(
            "partial", [P, TILES_PER_CORE], f32, isOutput=True
        )

    with ExitStack() as st:
        e = st.enter_context
        e_all = e(nc.sbuf_tensor("e_all", [P, TILES_PER_CORE, FEAT_DIM], bf16))
        c_all = e(nc.sbuf_tensor("c_all", [P, TILES_PER_CORE, FEAT_DIM], bf16))
        diff = e(nc.sbuf_tensor("diff", [P, TILES_PER_CORE, FEAT_DIM], bf16))
        sqscr = e(nc.sbuf_tensor("sqscr", [P, TILES_PER_CORE, FEAT_DIM], bf16))
        idx16 = e(nc.sbuf_tensor("idx16", [P, IDX_TOTAL_COLS], i16))
        rowtot = e(nc.sbuf_tensor("rowtot", [P, TILES_PER_CORE], f32))

        dma_idx = e(nc.semaphore("dma_idx"))
        dma_e = e(nc.semaphore("dma_e"))
        dma_ga = e(nc.semaphore("dma_ga"))
        dma_gb = e(nc.semaphore("dma_gb"))
        dma_out = e(nc.semaphore("dma_out"))
        prep_sem = e(nc.semaphore("prep_sem"))
        s_dve = e(nc.semaphore("s_dve"))
        s_diff = e(nc.semaphore("s_diff"))
        s_act = e(nc.semaphore("s_act"))

        block = e(nc.Block())

        @block.sync
        def _(sync: bass.BassEngine):
            sync.dma_start(out=idx16[:, :], in_=lab[:, :]).then_inc(dma_idx, 16)
            sync.dma_start(
                out=e_all[:], in_=emb.rearrange("(t p) d -> p t d", p=P)
            ).then_inc(dma_e, 16)
            if not USE_KV_OUT:
                sync.wait_ge(s_act, 1)
                sync.wait_ge(s_dve, 1)
                sync.dma_start(out=out[:, :], in_=rowtot[:]).then_inc(dma_out, 16)
                if reset_sems:
                    sync.sem_clear(s_dve)
                    sync.sem_clear(s_act)
                sync.wait_ge(dma_out, 16)
                if reset_sems:
                    sync.sem_clear(dma_out)

        @block.gpsimd
        def _(gpsimd: bass.BassGpSimd):
            if USE_KV_OUT:
                from concourse.library_config import attnmlp as glib
            else:
                from concourse.library_config import mlp as glib

            gpsimd.load_library(glib)
            gpsimd.wait_ge(dma_idx, 16)
            if USE_TRIGGER_GATHERS:
                gpsimd.dma_gather(
                    out_ap=c_all[:, 0:TILES_A, :],
                    in_ap=cen[:],
                    idxs_ap=idx16[:, 0:COLS_A],
                    num_idxs=ROWS_A,
                    num_idxs_reg=ROWS_A,
                    elem_size=FEAT_DIM,
                    prepare_only=True,
                    sem=dma_ga,
                ).then_inc(prep_sem, 1)
                gpsimd.dma_gather(
                    out_ap=c_all[:, TILES_A:TILES_PER_CORE, :],
                    in_ap=cen[:],
                    idxs_ap=idx16[:, COLS_A:IDX_COLS],
                    num_idxs=ROWS_B,
                    num_idxs_reg=ROWS_B,
                    elem_size=FEAT_DIM,
                    prepare_only=True,
                    sem=dma_gb,
                ).then_inc(prep_sem, 1)
                gpsimd.wait_ge(prep_sem, 1)
                gpsimd.trigger_dma(count=1)
                gpsimd.wait_ge(prep_sem, 2)
                gpsimd.trigger_dma(count=1)
            else:
                gpsimd.dma_gather(
                    out_ap=c_all[:, 0:TILES_A, :],
                    in_ap=cen[:],
                    idxs_ap=idx16[:, 0:COLS_A],
                    num_idxs=ROWS_A,
                    num_idxs_reg=ROWS_A,
                    elem_size=FEAT_DIM,
                ).then_inc(dma_ga, 16)
                gpsimd.dma_gather(
                    out_ap=c_all[:, TILES_A:TILES_PER_CORE, :],
                    in_ap=cen[:],
                    idxs_ap=idx16[:, COLS_A:IDX_COLS],
                    num_idxs=ROWS_B,
                    num_idxs_reg=ROWS_B,
                    elem_size=FEAT_DIM,
                ).then_inc(dma_gb, 16)
            if USE_KV_OUT:
                gpsimd.kv_writeback(
                    out_ap=out[:],
                    in_ap=rowtot[:].unsqueeze(-1).unsqueeze(-1),
                    ctx_idxs_ap=idx16[:, IDX_COLS : IDX_COLS + 2].bitcast(i32),
                    prepare_only=True,
                    sem=dma_out,
                ).then_inc(prep_sem, 1)
                gpsimd.wait_ge(prep_sem, 3 if USE_TRIGGER_GATHERS else 1)
                gpsimd.wait_ge(s_act, 1)
                gpsimd.wait_ge(s_dve, 1)
                gpsimd.trigger_dma(count=1)
                if reset_sems:
                    gpsimd.sem_clear(s_dve)
                    gpsimd.sem_clear(s_act)
                gpsimd.wait_ge(dma_out, 16)
                if reset_sems:
                    gpsimd.sem_clear(dma_out)
            if reset_sems:
                gpsimd.sem_clear(dma_idx)
                if USE_TRIGGER_GATHERS or USE_KV_OUT:
                    gpsimd.sem_clear(prep_sem)

        @block.vector
        def _(vector: bass.BassEngine):
            vector.wait_ge(dma_e, 16)
            vector.wait_ge(dma_ga, 16)
            # diff = c - e over tiles 0-2 in one 2x-mode bf16 op, then
            # square + row-reduce per tile via scalar_tensor_tensor (pow with
            # a reduce accumulator is invalid ISA; stt runs modeless at 1x).
            # ACT squares tile 1 in parallel to shorten the DVE tail.
            vector.tensor_tensor(
                out=diff[:, 0:TILES_A, :],
                in0=c_all[:, 0:TILES_A, :],
                in1=e_all[:, 0:TILES_A, :],
                op=Alu.subtract,
            ).then_inc(s_diff, 1)
            for t in (0, 2):
                vector.scalar_tensor_tensor(
                    out=sqscr[:, t, :],
                    in0=diff[:, t, :],
                    scalar=1.0,
                    in1=diff[:, t, :],
                    op0=Alu.mult,
                    op1=Alu.mult,
                    accum_out=rowtot[:, t : t + 1],
                )
            vector.wait_ge(dma_gb, 16)
            vector.tensor_tensor(
                out=diff[:, 3, :],
                in0=c_all[:, 3, :],
                in1=e_all[:, 3, :],
                op=Alu.subtract,
            )
            vector.scalar_tensor_tensor(
                out=sqscr[:, 3, :],
                in0=diff[:, 3, :],
                scalar=1.0,
                in1=diff[:, 3, :],
                op0=Alu.mult,
                op1=Alu.mult,
                accum_out=rowtot[:, 3:4],
            ).then_inc(s_dve, 1)
            if reset_sems:
                vector.sem_clear(dma_e)
                vector.sem_clear(dma_ga)
                vector.sem_clear(dma_gb)

        @block.scalar
        def _(scalar: bass.BassEngine):
            scalar.wait_ge(s_diff, 1)
            scalar.activation(
                out=sqscr[:, 1, :],
                in_=diff[:, 1, :],
                func=mybir.ActivationFunctionType.Square,
                accum_out=rowtot[:, 1:2],
            ).then_inc(s_act, 1)
            if reset_sems:
                scalar.sem_clear(s_diff)

    nc.compile()
    return nc


def _get_nc() -> bass.Bass:
    global _nc_cache
    if _nc_cache is None:
        _nc_cache = build_bass()
    return _nc_cache


def make_in_maps(embeddings, labels, centers):
    embeddings = np.ascontiguousarray(embeddings, dtype=np.float32).astype(
        ml_dtypes.bfloat16
    )
    labels = np.asarray(labels)
    centers = np.ascontiguousarray(centers, dtype=np.float32).astype(
        ml_dtypes.bfloat16
    )
    in_maps = []
    for c in range(N_CORES):
        s = slice(c * ROWS_PER_CORE, (c + 1) * ROWS_PER_CORE)
        wrap16 = labels[s].astype(np.int16).reshape(IDX_COLS, IDX_WRAP).T
        lab_wrapped = np.tile(wrap16, (P // IDX_WRAP, 1))
        lab_full = np.zeros((P, IDX_TOTAL_COLS), dtype=np.int16)
        lab_full[:, :IDX_COLS] = lab_wrapped
        in_maps.append(
            {
                "embeddings": embeddings[s],
                "labels": np.ascontiguousarray(lab_full),
                "centers": centers,
            }
        )
    return in_maps


def run(embeddings, labels, centers, **run_kwargs):
    import time

    from concourse.bass_utils import run_bass_kernel_spmd

    nc = _get_nc()
    in_maps = make_in_maps(embeddings, labels, centers)
    try:
        res = run_bass_kernel_spmd(nc, in_maps, list(range(N_CORES)), **run_kwargs)
    except Exception:
        # one retry for transient runtime/worker hiccups
        time.sleep(5)
        res = run_bass_kernel_spmd(nc, in_maps, list(range(N_CORES)), **run_kwargs)
    dists = np.asarray(
        [res.results[c]["partial"].reshape(P, TILES_PER_CORE) for c in range(N_CORES)],
        dtype=np.float64,
    )
    clipped = np.clip(dists, CLAMP_MIN, CLAMP_MAX)
    loss = clipped.sum() / BATCH + (NUM_CLASSES - 1) * CLAMP_MIN
    return np.float32(loss * LAMBDA_C), res


def kernel(embeddings, labels, centers):
    loss, _ = run(embeddings, labels, centers)
    return loss
